# revision 42
# baseline (speedup 1.0000x reference)
"""MultiHeadAttentionBlock (B=2, S=2048, D=1024, H=16, causal) on 8 trn2 cores.

Sharding: tensor-parallel over heads (2 heads / core) for QKV projections and
attention. The context is redistributed with FOUR per-(batch, i-half) quarter
AllToAlls so the output projection starts while later attention quarters are
still running; each core owns a 128-row block of every quarter (4 x 128 = 512
output rows / core). The host only slices / transposes / casts inputs and
reassembles the 32 row-blocks (output is bf16 on-device, upcast on host).

Per-core dataflow (all matmuls bf16 with fp32 PSUM accumulation):
  QT_b = (w_q x_b)^T  [128, 2048]  (dims on partitions; bias on DVE evict)
  KT_b = same; V_b in natural layout via PE transpose of VT; a ones-column is
  appended per head so PV accumulates the softmax denominator for free.
  Phase order (b, hl, ih): both head-halves of a quarter run back-to-back so
  quarter q=2b+ih finishes early; its AllToAll fires immediately and the
  o-proj unit for q becomes PE filler inside later exp-paced phases.
  S^T[j,i] = K_j^T Q_i per (batch, head), causal blocks only; exp on ScalarE
  (scale=1/8, no max-subtraction needed); triangular mask on the diagonal
  block via VectorE. PV is TRANSPOSED: the ex block is the stationary operand
  and [V_h | 1] streams, so context accumulates as [i-rows, dk+1] and the
  denominator lands per-partition -- normalize is a [128,1] reciprocal +
  tensor_scalar multiply (no partition broadcast). PV is emitted one jt late
  (software pipeline) so its 8 waiters never clog the 4-deep PE wait queue.
  PSUM allows one accumulation group per 2KB bank, so the 4 i-block slices
  packed per ctx tile share one start/stop group and normalize batches at
  group close. Context sends are per-i-block 2-D DMAs (3-D SBUF-side
  rearranges corrupt data on this stack), destination core = i-block index;
  tail sends ride the idle ACT/SP queues. x loads are one 1MB DMA per
  512-row proj chunk issued a phase ahead in deadline order; ring depths are
  sized so a phase never waits on the previous phase's send transfers. A
  paced PE trickle spans the final collective so the last o-proj unit runs
  at the warm p-state.
"""

import os
import numpy as np
import ml_dtypes

B, S, D = 2, 2048, 1024
H, DK = 16, 64
ROWS = B * S  # 4096
NCORES = 8
CDIM = 128  # context dims per core (2 heads x 64)
RPC = ROWS // NCORES  # 512 output rows per core (4 quarters x 128)
NQ = 4  # quarters: q = 2*b + ih
CPQ = 128  # rows per core per quarter

BF16 = ml_dtypes.bfloat16

_CACHE = {}
LAST_RESULTS = None  # stashed BassKernelResults for external inspection


def _build_program(with_collective=True):
    import concourse.mybir as mybir
    import concourse.tile as tile
    from concourse import bacc
    from concourse.masks import make_identity

    f32 = mybir.dt.float32
    bf = mybir.dt.bfloat16
    Exp = mybir.ActivationFunctionType.Exp

    nc = bacc.Bacc(
        "TRN2", target_bir_lowering=False, debug=False, num_devices=NCORES
    )

    # --- per-core DRAM I/O ---
    xqT_d = nc.dram_tensor("xqT", [D, ROWS], bf, kind="ExternalInput").ap()
    xkT_d = nc.dram_tensor("xkT", [D, ROWS], bf, kind="ExternalInput").ap()
    xvT_d = nc.dram_tensor("xvT", [D, ROWS], bf, kind="ExternalInput").ap()
    wqT_d = nc.dram_tensor("wqT", [D, CDIM], bf, kind="ExternalInput").ap()
    wkT_d = nc.dram_tensor("wkT", [D, CDIM], bf, kind="ExternalInput").ap()
    wvT_d = nc.dram_tensor("wvT", [D, CDIM], bf, kind="ExternalInput").ap()
    bq_d = nc.dram_tensor("bq", [CDIM, 1], f32, kind="ExternalInput").ap()
    bk_d = nc.dram_tensor("bk", [CDIM, 1], f32, kind="ExternalInput").ap()
    bv_d = nc.dram_tensor("bv", [CDIM, 1], f32, kind="ExternalInput").ap()
    woT_d = nc.dram_tensor("woT", [D, D], bf, kind="ExternalInput").ap()
    bo_d = nc.dram_tensor("bo", [1, D], bf, kind="ExternalInput").ap()
    triu_d = nc.dram_tensor("triu", [128, 128], bf, kind="ExternalInput").ap()
    out_d = nc.dram_tensor("out", [RPC, D], bf, kind="ExternalOutput").ap()

    xd = {"q": xqT_d, "k": xkT_d, "v": xvT_d}

    with tile.TileContext(nc) as tc:
        with (
            tc.tile_pool(name="sb", bufs=1) as sb,
            tc.tile_pool(name="ps", bufs=1, space="PSUM") as ps,
            tc.tile_pool(name="dram", bufs=1, space="DRAM") as dram,
        ):
            # --- weights / consts (issued in need order) ---
            wq3 = sb.tile([128, 8, CDIM], bf, tag="w", bufs=3)
            nc.sync.dma_start(
                out=wq3, in_=wqT_d.rearrange("(ko ki) m -> ki ko m", ki=128)
            )
            bq_sb = sb.tile([CDIM, 1], f32, tag="bias", bufs=3)
            bk_sb = sb.tile([CDIM, 1], f32, tag="bias", bufs=3)
            wk3 = sb.tile([128, 8, CDIM], bf, tag="w", bufs=3)
            wv3 = sb.tile([128, 8, CDIM], bf, tag="w", bufs=3)
            bv_sb = sb.tile([CDIM, 1], f32, tag="bias", bufs=3)
            bo_sb = sb.tile([1, D], bf, tag="bo", bufs=1)
            triu_sb = sb.tile([128, 128], bf, tag="triu", bufs=1)
            ones_sb = sb.tile([1, 128], bf, tag="ones", bufs=1)
            nc.vector.memset(ones_sb, 1.0)
            ident_sb = sb.tile([128, 128], bf, tag="ident", bufs=1)
            make_identity(nc, ident_sb)
            # preload the exp table set during the DMA ramp so the first real
            # exp doesn't pay the ~1.3us ACT_TABLE_LOAD
            warm_sb = sb.tile([128, 128], bf, tag="warm", bufs=1)
            nc.vector.memset(warm_sb, 1.0)
            nc.scalar.activation(
                out=warm_sb[0:1, 0:1], in_=ones_sb[0:1, 0:1], func=Exp, scale=1.0
            )
            wo3 = sb.tile([128, 8, D], bf, tag="wo", bufs=1)

            send_q = [
                dram.tile([NCORES, CDIM, CPQ], bf, tag=f"snd{q}", name=f"send{q}")
                for q in range(NQ)
            ]
            recv_q = [
                dram.tile([NCORES, CDIM, CPQ], bf, tag=f"rcv{q}", name=f"recv{q}")
                for q in range(NQ)
            ]

            # per-batch persistent tiles
            QT, KT, VT, V3 = {}, {}, {}, {}
            for b in range(B):
                QT[b] = sb.tile([128, S], bf, tag="qt", bufs=2, name=f"QT{b}")
                KT[b] = sb.tile([128, S], bf, tag="kt", bufs=2, name=f"KT{b}")
                VT[b] = sb.tile([128, S], bf, tag="vt", bufs=2, name=f"VT{b}")
                V3[b] = sb.tile([128, 16, 130], bf, tag="v3", bufs=2, name=f"V3{b}")
                nc.vector.memset(V3[b][:, :, 64:65], 1.0)
                nc.vector.memset(V3[b][:, :, 129:130], 1.0)

            xts = {}  # (pref, b, n) -> loaded [128, 8, 512] chunk

            def load_xc(pref, b, n, split=1):
                # 1MB for rows [512n, 512n+512) of batch b (2KB-contig runs);
                # split>1 issues sub-DMAs along the contraction dim so the
                # first proj matmuls start before the full chunk lands
                t = sb.tile(
                    [128, 8, 512], bf, tag="xt", bufs=5, name=f"x{pref}{b}_{n}"
                )
                co = S * b + 512 * n
                kstep = 8 // split
                for g in range(split):
                    nc.sync.dma_start(
                        out=t[:, g * kstep : (g + 1) * kstep, :],
                        in_=xd[pref][
                            128 * kstep * g : 128 * kstep * (g + 1), co : co + 512
                        ].rearrange("(ko ki) m -> ki ko m", ki=128),
                    )
                xts[(pref, b, n)] = t

            W3 = {"q": wq3, "k": wk3, "v": wv3}
            BS = {"q": bq_sb, "k": bk_sb, "v": bv_sb}
            OT = {"q": QT, "k": KT, "v": VT}

            def proj(pref, b, n):
                # one 512-row chunk of a projection, contraction over D
                xt = xts[(pref, b, n)]
                pt = ps.tile([128, 512], f32, tag="proj", bufs=2, name="pproj")
                for kk in range(8):
                    nc.tensor.matmul(
                        pt,
                        W3[pref][:, kk, :],
                        xt[:, kk, :],
                        start=(kk == 0),
                        stop=(kk == 7),
                    )
                nc.vector.tensor_scalar_add(
                    out=OT[pref][b][:, n * 512 : (n + 1) * 512],
                    in0=pt,
                    scalar1=BS[pref],
                )

            def v_tile(b, rt):
                # transpose one [128, 128] tile of VT into natural layout; per
                # head laid out [V_h | ones] in V3 (ones cols preset).
                pv = ps.tile([128, 128], bf, tag="proj", bufs=2, name="pvt")
                nc.tensor.transpose(
                    pv, VT[b][:, rt * 128 : (rt + 1) * 128], ident_sb
                )
                nc.vector.tensor_copy(out=V3[b][:, rt, 0:64], in_=pv[:, 0:64])
                nc.vector.tensor_copy(out=V3[b][:, rt, 65:129], in_=pv[:, 64:128])

            rcvt = {}

            def rcv(q):
                # recv quarter q into SBUF in two halves so the o-proj's
                # first accumulations start after half the transfer
                t = sb.tile([128, NCORES, CPQ], bf, tag="rcvt", bufs=2, name=f"rc{q}")
                nc.sync.dma_start(
                    out=t[:, 0:4, :], in_=recv_q[q][0:4].rearrange("p c r -> c p r")
                )
                nc.sync.dma_start(
                    out=t[:, 4:8, :], in_=recv_q[q][4:8].rearrange("p c r -> c p r")
                )
                rcvt[q] = t

            def oproj_half(q, half):
                # 512 output cols of the 128-row o-proj unit for quarter q
                po = ps.tile([128, 512], f32, tag="proj", bufs=2, name=f"po{q}{half}")
                cs = slice(512 * half, 512 * half + 512)
                nc.tensor.matmul(po, ones_sb, bo_sb[:, cs], start=True, stop=False)
                for p in range(NCORES):
                    nc.tensor.matmul(
                        po, rcvt[q][:, p, :], wo3[:, p, cs], start=False, stop=(p == 7)
                    )
                ob = sb.tile([128, 512], bf, tag="ob", bufs=2, name=f"ob{q}{half}")
                nc.vector.tensor_copy(out=ob, in_=po)
                nc.sync.dma_start(out=out_d[q * CPQ : (q + 1) * CPQ, cs], in_=ob)

            def attention(b, hl, ih, fillers):
                # fillers: jt -> list of thunks (later-phase PE work) injected
                # so TensorE stays fed while ScalarE paces the exp stream.
                # PV is transposed: ex blocks are the stationary operand and
                # V3 streams, so the context accumulates as [i-rows, dk+1] —
                # the softmax denominator lands per-partition and normalize
                # is a [128,1] reciprocal + tensor_scalar multiply.
                pb = 64 * hl
                ibase = 1024 * ih
                q = 2 * b + ih
                ctx = {
                    g: ps.tile([128, 4, 65], f32, tag="ctx", bufs=2,
                               name=f"ctx{b}{hl}{ih}{g}")
                    for g in (0, 1)
                }
                # staging for this phase's transposed+normalized context
                # [64 dims, 1024 rows], sent as one DMA at phase end
                cstg = sb.tile([128, 1024], bf, tag="cstg", bufs=4, name="cstg")
                def pv_batch(jt, ex):
                    # PV per i-block: ctx[ib] += ex_blk^T @ [V_h | 1].
                    # Emitted one jt late (software pipeline) so these PE
                    # instructions arrive after their exp already completed —
                    # otherwise 8 waiters clog the 4-deep PE wait queue.
                    # PSUM allows ONE accumulation group per 2KB bank: the 4
                    # i-block slices packed in a ctx tile share a group —
                    # start on the group's first matmul (marks the whole bank
                    # pending-zero), stop on its last (ibl 4g+3 finishing)
                    for ibl in range(max(0, jt - 8 * ih), 8):
                        nc.tensor.matmul(
                            ctx[ibl // 4][:, ibl % 4, :],
                            ex[:, 128 * ibl : 128 * ibl + 128],
                            V3[b][:, jt, 65 * hl : 65 * hl + 65],
                            start=(jt == 0 and ibl % 4 == 0),
                            stop=(jt == ibl + 8 * ih and ibl % 4 == 3),
                        )
                    gdone = jt - 8 * ih
                    if 0 <= gdone < 8 and gdone % 4 == 3:
                        # bank group g closed: normalize its 4 i-blocks (PSUM
                        # col 64 holds the denominator per row), transpose to
                        # [dims, rows] and stage for the quarter's AllToAll.
                        # PSUM can't be read while its bank's accumulation
                        # group is open, so normalizes batch at group close.
                        for ibl in range(gdone - 3, gdone + 1):
                            cx = ctx[ibl // 4][:, ibl % 4, :]
                            rs = sb.tile([128, 1], f32, tag="rs", bufs=8, name="rs")
                            nc.vector.reciprocal(out=rs, in_=cx[:, 64:65])
                            cn = sb.tile([128, 64], bf, tag="cn", bufs=8, name="cn")
                            nc.vector.tensor_scalar_mul(
                                out=cn, in0=cx[:, 0:64], scalar1=rs
                            )
                            ct = ps.tile([64, 128], bf, tag="proj", bufs=2, name="ct")
                            nc.tensor.transpose(ct, cn, ident_sb)
                            nc.vector.tensor_copy(
                                out=cstg[0:64, 128 * ibl : 128 * ibl + 128],
                                in_=ct,
                            )

                pend = None  # (jt, ex) whose PV batch is not yet emitted
                for jt in range(8 * (ih + 1)):
                    for f in fillers.get(jt, ()):
                        f()
                    jpos = 128 * jt
                    if hl == 0 and jt // 8 == ih:
                        v_tile(b, jt)
                    estart = max(jpos, ibase)
                    off0 = estart - ibase
                    ex = sb.tile([128, 1024], bf, tag="ex", bufs=8, name="ex")
                    sc = ps.tile([128, 1024], f32, tag="sc", bufs=2, name="sc")
                    split_exp = b == 0 and hl == 0 and ih == 0 and jt == 0
                    off = off0
                    while off < 1024:
                        cw = min(512 - off % 512, 1024 - off)
                        nc.tensor.matmul(
                            sc[:, off : off + cw],
                            KT[b][pb : pb + 64, jpos : jpos + 128],
                            QT[b][pb : pb + 64, ibase + off : ibase + off + cw],
                            start=True,
                            stop=True,
                        )
                        off += cw
                        if split_exp:
                            nc.scalar.activation(
                                out=ex[:, off - cw : off],
                                in_=sc[:, off - cw : off],
                                func=Exp,
                                scale=0.125,
                            )
                    if not split_exp:
                        nc.scalar.activation(
                            out=ex[:, off0:1024],
                            in_=sc[:, off0:1024],
                            func=Exp,
                            scale=0.125,
                        )
                    if jt // 8 == ih:
                        # diagonal block lives in this i-half: mask it
                        dg = jpos - ibase
                        nc.vector.tensor_mul(
                            ex[:, dg : dg + 128], ex[:, dg : dg + 128], triu_sb
                        )
                    if pend is not None:
                        pv_batch(*pend)
                    pend = (jt, ex)
                pv_batch(*pend)
                # one send for this phase's 64-dim half of quarter q
                nc.gpsimd.dma_start(
                    out=send_q[q][:, pb : pb + 64, :],
                    in_=cstg[0:64, :].rearrange("p (c r) -> c p r", c=8),
                )

            def a2a(q):
                if with_collective:
                    nc.gpsimd.collective_compute(
                        "AllToAll",
                        mybir.AluOpType.bypass,
                        replica_groups=[list(range(NCORES))],
                        ins=[send_q[q].opt()],
                        outs=[recv_q[q].opt()],
                    )
                else:
                    # timing-only stand-in (TimelineSim has no collectives)
                    nc.gpsimd.dma_start(out=recv_q[q], in_=send_q[q])

            L = lambda pref, b, n: (lambda: load_xc(pref, b, n))
            P = lambda pref, b, n: (lambda: proj(pref, b, n))
            U = lambda q, half: (lambda: oproj_half(q, half))

            def Lwo():
                nc.sync.dma_start(
                    out=wo3, in_=woT_d.rearrange("(ko ki) m -> ki ko m", ki=128)
                )

            def sched(*pairs):
                # pairs of (jt, thunk) -> fillers dict
                d = {}
                for jt, t in pairs:
                    d.setdefault(jt, []).append(t)
                return d

            # --- prologue: minimal data for attention(0,0,0); the load
            # stream is ordered by global consumption deadline so the DMA
            # engine (the pacer of the front half) never delivers late ---
            nc.sync.dma_start(
                out=wk3, in_=wkT_d.rearrange("(ko ki) m -> ki ko m", ki=128)
            )
            load_xc("k", 0, 0, split=2)
            nc.sync.dma_start(out=bq_sb, in_=bq_d)
            nc.sync.dma_start(out=bk_sb, in_=bk_d)
            load_xc("q", 0, 0, split=2)
            proj("k", 0, 0)
            load_xc("q", 0, 1)
            proj("q", 0, 0)
            proj("q", 0, 1)
            nc.sync.dma_start(
                out=wv3, in_=wvT_d.rearrange("(ko ki) m -> ki ko m", ki=128)
            )
            load_xc("v", 0, 0, split=2)
            nc.sync.dma_start(out=bv_sb, in_=bv_d)
            nc.sync.dma_start(out=triu_sb, in_=triu_d)
            nc.sync.dma_start(out=bo_sb, in_=bo_d)
            proj("v", 0, 0)
            load_xc("v", 0, 1)  # v first: longer chain (proj+transpose+copy)
            load_xc("k", 0, 1)
            load_xc("q", 0, 2)
            load_xc("q", 0, 3)

            # --- phases; fillers keep PE fed under the ScalarE exp stream.
            # K/V chunk n feeds scores/v_tile from jt=4n of its quarter; all
            # loads are issued a phase (or more) ahead at early jt slots. ---
            attention(0, 0, 0, sched(
                (0, L("k", 0, 2)), (1, L("v", 0, 2)),
                (3, P("v", 0, 1)), (4, P("k", 0, 1)),
            ))
            attention(0, 1, 0, sched(
                (0, L("k", 0, 3)), (1, L("v", 0, 3)),
                (2, P("q", 0, 2)), (4, P("q", 0, 3)),
            ))
            a2a(0)
            rcv(0)
            attention(0, 0, 1, sched(
                (0, L("q", 1, 0)), (1, L("q", 1, 1)),
                (2, P("k", 0, 2)), (4, P("v", 0, 2)),
                (5, Lwo), (6, L("k", 1, 0)), (7, L("v", 1, 0)),
                (8, P("k", 0, 3)), (10, P("v", 0, 3)),
            ))
            attention(0, 1, 1, sched(
                (0, L("k", 1, 1)), (1, L("v", 1, 1)),
                (2, P("q", 1, 0)), (4, P("q", 1, 1)),
                (6, L("q", 1, 2)), (7, L("q", 1, 3)),
                (8, U(0, 0)), (10, P("k", 1, 0)), (12, P("v", 1, 0)),
                (13, L("k", 1, 2)), (15, L("v", 1, 2)),
                (14, U(0, 1)),
            ))
            a2a(1)
            rcv(1)
            attention(1, 0, 0, sched(
                (2, P("k", 1, 1)), (3, P("v", 1, 1)),
                (4, L("k", 1, 3)), (5, L("v", 1, 3)),
                (5, P("k", 1, 2)), (6, P("v", 1, 2)),
            ))
            attention(1, 1, 0, sched(
                (1, P("q", 1, 2)), (3, P("q", 1, 3)),
                (5, P("k", 1, 3)), (7, P("v", 1, 3)),
            ))
            a2a(2)
            rcv(2)
            attention(1, 0, 1, sched(
                (1, U(1, 0)), (5, U(1, 1)), (10, U(2, 0)),
            ))
            attention(1, 1, 1, sched(
                (2, U(2, 1)),
            ))
            a2a(3)
            rcv(3)
            # keep the PE p-state warm across the final collective wait so
            # the last o-proj unit runs at full clock
            for _ in range(12):
                pwk = ps.tile([128, 128], f32, tag="proj", bufs=2, name="pwk")
                nc.tensor.matmul(
                    pwk, ones_sb, warm_sb[0:1, :], start=True, stop=True
                )
                nc.vector.tensor_copy(out=warm_sb, in_=pwk)
            oproj_half(3, 0)
            oproj_half(3, 1)

    nc.compile()
    return nc


def _prep_inputs(q, k, v, w_q, b_q, w_k, b_k, w_v, b_v, w_o, b_o):
    def bf(x):
        return np.ascontiguousarray(x).astype(BF16)

    q = np.asarray(q, np.float32).reshape(ROWS, D)
    k = np.asarray(k, np.float32).reshape(ROWS, D)
    v = np.asarray(v, np.float32).reshape(ROWS, D)
    xqT = bf(q.T)
    xkT = bf(k.T)
    xvT = bf(v.T)
    w_q = np.asarray(w_q, np.float32)
    w_k = np.asarray(w_k, np.float32)
    w_v = np.asarray(w_v, np.float32)
    w_o = np.asarray(w_o, np.float32)
    woT = bf(w_o.T)
    bo = bf(np.asarray(b_o, np.float32).reshape(1, D))
    triu = np.triu(np.ones((128, 128), np.float32)).astype(BF16)

    in_maps = []
    for c in range(NCORES):
        hs = slice(c * CDIM, (c + 1) * CDIM)
        in_maps.append(
            {
                "xqT": xqT,
                "xkT": xkT,
                "xvT": xvT,
                "wqT": bf(w_q[hs, :].T),
                "wkT": bf(w_k[hs, :].T),
                "wvT": bf(w_v[hs, :].T),
                "bq": np.ascontiguousarray(
                    np.asarray(b_q, np.float32)[hs].reshape(CDIM, 1)
                ),
                "bk": np.ascontiguousarray(
                    np.asarray(b_k, np.float32)[hs].reshape(CDIM, 1)
                ),
                "bv": np.ascontiguousarray(
                    np.asarray(b_v, np.float32)[hs].reshape(CDIM, 1)
                ),
                "woT": woT,
                "bo": bo,
                "triu": triu,
            }
        )
    return in_maps


def kernel(q, k, v, mask, w_q, b_q, w_k, b_k, w_v, b_v, w_o, b_o):
    global LAST_RESULTS
    if "nc" not in _CACHE:
        _CACHE["nc"] = _build_program()
    nc = _CACHE["nc"]

    from concourse.bass_utils import run_bass_kernel_spmd

    in_maps = _prep_inputs(q, k, v, w_q, b_q, w_k, b_k, w_v, b_v, w_o, b_o)
    res = run_bass_kernel_spmd(nc, in_maps, core_ids=list(range(NCORES)))
    LAST_RESULTS = res
    # core c's out rows: 4 quarters q=2b+ih, each 128 rows at global
    # batch b, rows [1024*ih + 128*c, +128)
    out = np.empty((B, S, D), np.float32)
    for c in range(NCORES):
        oc = np.asarray(res.results[c]["out"], np.float32)
        for qtr in range(NQ):
            b, ih = qtr // 2, qtr % 2
            r0 = 1024 * ih + CPQ * c
            out[b, r0 : r0 + CPQ, :] = oc[qtr * CPQ : (qtr + 1) * CPQ, :]
    return out
